# revision 7
# baseline (speedup 1.0000x reference)
"""Deformable conv (nn_DeformConv) Trainium2 Bass kernel.

Strategy (per core = one batch of 8, data-parallel):
  1. 1x1 conv (PE) + depthwise 3x3 (DVE, shifted views) -> offsets [18, 4096]
  2. PE-transpose offsets to position-partition layout; batched per-position
     floor/residual math and flat gather index r0 into a zero-padded 72x72 grid.
  3. DRAM table [5248 rows, 1024] bf16, row r = [x[r] | Dx[r] | Dy[r] | Dxy[r]]
     (finite differences of zero-padded x). Bilinear sample ==
     x[r0] + rx*Dx[r0] + ry*Dy[r0] + rx*ry*Dxy[r0] (exact, incl. OOB zeroing).
  4. Per 128-position tile: indirect row-gathers (per tap), wide broadcast-AP
     multiply + 3 adds for the combine, PE-transpose sampled into 18 ck-tiles,
     PSUM-accumulated matmul against w_def (bf16), DMA out.
"""
import os
import numpy as np
from contextlib import ExitStack

import concourse.bass as bass
import concourse.mybir as mybir
import concourse.tile as tile
from concourse import bacc as _bacc
from concourse.bass import IndirectOffsetOnAxis
from concourse.masks import make_identity

FP32 = mybir.dt.float32
BF16 = mybir.dt.bfloat16
I32 = mybir.dt.int32
I16 = mybir.dt.int16

N, C, H, W = 8, 256, 64, 64
HW = H * W                    # 4096
K = 9
OFFC = 18
PAD = 4
G = H + 2 * PAD               # 72
ROWS = G * G                  # 5184
RT = 5248                     # rows padded to 41*128
NRT = RT // 128               # 41
NPT = HW // 128               # 32 position tiles
CT = C // 128                 # 2 channel tiles
KT = (C * K) // 128           # 18 contraction tiles
ALU = mybir.AluOpType
AF = mybir.ActivationFunctionType

MODE = os.environ.get("MODE", "full")   # full | pre
# NOTE: multi-offset indirect DMA fails at runtime on HW; keep per-tap gathers.
MERGE_GATHER = os.environ.get("MERGE_GATHER", "0") == "1"


def build_nc():
    nc = _bacc.Bacc()
    x_d = nc.dram_tensor("x", [C, HW], FP32, kind="ExternalInput")
    w_adj_d = nc.dram_tensor("w_adj", [OFFC, C], FP32, kind="ExternalInput")
    b_adj_d = nc.dram_tensor("b_adj", [OFFC, 1], FP32, kind="ExternalInput")
    w_off_d = nc.dram_tensor("w_off", [OFFC, K], FP32, kind="ExternalInput")
    b_off_d = nc.dram_tensor("b_off", [OFFC, 1], FP32, kind="ExternalInput")
    w_def_d = nc.dram_tensor("w_def", [C, C * K], FP32, kind="ExternalInput")
    out_d = nc.dram_tensor("out", [C, HW], FP32, kind="ExternalOutput")

    with tile.TileContext(nc) as tc, ExitStack() as ctx:
        pers = ctx.enter_context(tc.tile_pool(name="pers", bufs=1))
        dram = ctx.enter_context(tc.tile_pool(name="dram", bufs=1, space="DRAM"))

        table = dram.tile([RT, 4 * C], BF16)

        ident_f = pers.tile([128, 128], FP32)
        make_identity(nc, ident_f[:])
        ident_b = pers.tile([128, 128], BF16)
        nc.vector.tensor_copy(ident_b[:], ident_f[:])

        # per-partition constants: hh = p//64 (0/1), ww = p%64
        iota_p = pers.tile([128, 1], I32)
        nc.gpsimd.iota(iota_p[:], pattern=[[0, 1]], base=0, channel_multiplier=1)
        pf = pers.tile([128, 1], FP32)
        nc.vector.tensor_copy(pf[:], iota_p[:])
        hh = pers.tile([128, 1], FP32)
        nc.vector.tensor_scalar(out=hh[:], in0=pf[:], scalar1=64.0, scalar2=None,
                                op0=ALU.is_ge)
        ww = pers.tile([128, 1], FP32)
        nc.vector.scalar_tensor_tensor(out=ww[:], in0=hh[:], scalar=-64.0,
                                       in1=pf[:], op0=ALU.mult, op1=ALU.add)

        # batched base ramps over (t, k): by = 2t + ki + (PAD-1), bx = kj + (PAD-1)
        by_i = pers.tile([128, NPT, K], I32)
        nc.gpsimd.iota(by_i[:], pattern=[[2, NPT], [1, 3], [0, 3]], base=PAD - 1,
                       channel_multiplier=0)
        bx_i = pers.tile([128, NPT, K], I32)
        nc.gpsimd.iota(bx_i[:], pattern=[[0, NPT], [0, 3], [1, 3]], base=PAD - 1,
                       channel_multiplier=0)
        by_f = pers.tile([128, NPT, K], FP32)
        nc.vector.tensor_copy(by_f[:], by_i[:])
        bx_f = pers.tile([128, NPT, K], FP32)
        nc.vector.tensor_copy(bx_f[:], bx_i[:])

        w_defT = pers.tile([128, KT, 2 * 128], BF16)   # [ck-part, kt, o]
        # gather indices, 16-partition-wrapped for dma_gather:
        # idx16[p, t, 8k+r] = r0(q=16r+p%16, t, k)
        idx16 = pers.tile([128, NPT, K, 8], I16)
        wts_sb = pers.tile([128, NPT, K * 3], FP32)    # k-major (rx, ry, rxry)

        # ---------------- phase 3: w_def transpose (overlaps phase 1) ----------------
        xs_stack = ExitStack()
        xp = xs_stack.enter_context(tc.tile_pool(name="xp", bufs=1))
        with tc.tile_pool(name="psW", bufs=4, space="PSUM") as psW:
            w_def_sb = xp.tile([128, 2, C * K], FP32)
            for ot in range(2):
                nc.sync.dma_start(out=w_def_sb[:, ot, :],
                                  in_=w_def_d[ot * 128:(ot + 1) * 128, :])
            for kt in range(KT):
                k = kt // 2
                chalf = kt % 2
                for ot in range(2):
                    ps = psW.tile([128, 128], FP32, tag="psw")
                    src = w_def_sb[:, ot, :].rearrange("p (c k) -> p k c", k=K) \
                        [:, k, chalf * 128:(chalf + 1) * 128]
                    nc.tensor.transpose(ps[:], src, ident_f[:])
                    nc.scalar.copy(w_defT[:, kt, ot * 128:ot * 128 + 128], ps[:])

        # ---------------- phase 1: offsets pipeline ----------------
        x_sb = xp.tile([128, CT, HW], FP32)
        for ct in range(CT):
            nc.sync.dma_start(out=x_sb[:, ct, :], in_=x_d[ct * 128:(ct + 1) * 128, :])

        ph1 = ExitStack()
        offp = ph1.enter_context(tc.tile_pool(name="offp", bufs=1))
        psA = ph1.enter_context(tc.tile_pool(name="psA", bufs=2, space="PSUM"))

        w_adjT = offp.tile([128, CT, OFFC], FP32)
        for ct in range(CT):
            nc.sync.dma_start(
                out=w_adjT[:, ct, :],
                in_=w_adj_d.rearrange("o c -> c o")[ct * 128:(ct + 1) * 128, :])
        b_adj_sb = offp.tile([OFFC, 1], FP32)
        nc.sync.dma_start(out=b_adj_sb[:], in_=b_adj_d[:, :])
        w_off_sb = offp.tile([OFFC, K], FP32)
        nc.sync.dma_start(out=w_off_sb[:], in_=w_off_d[:, :])
        b_off_sb = offp.tile([OFFC, 1], FP32)
        nc.sync.dma_start(out=b_off_sb[:], in_=b_off_d[:, :])

        # 1x1 conv -> x_chan (padded 66x66 for the depthwise conv)
        GC = H + 2   # 66
        xch_pad = offp.tile([OFFC, GC * GC], BF16)
        nc.scalar.memzero(xch_pad[:])
        xch_v = xch_pad[:].rearrange("p (h w) -> p h w", h=GC, w=GC)
        for pch in range(8):
            ps = psA.tile([OFFC, 512], FP32)
            for ct in range(CT):
                nc.tensor.matmul(out=ps[:], lhsT=w_adjT[:, ct, :],
                                 rhs=x_sb[:, ct, pch * 512:(pch + 1) * 512],
                                 start=(ct == 0), stop=(ct == CT - 1))
            nc.scalar.activation(
                out=xch_v[:, 1 + pch * 8:1 + pch * 8 + 8, 1:1 + W],
                in_=ps[:].rearrange("p (h w) -> p h w", h=8, w=W),
                func=AF.Identity, bias=b_adj_sb[:], scale=1.0)

        # depthwise 3x3 -> offsets [18, 4096] (DVE chain, bf16)
        off_sb = offp.tile([OFFC, HW], BF16)
        ova = off_sb[:].rearrange("p (h w) -> p h w", h=H, w=W)
        for tap in range(K):
            di, dj = tap // 3, tap % 3
            vin = xch_v[:, di:di + H, dj:dj + W]
            if tap == 0:
                nc.vector.tensor_scalar(
                    out=ova, in0=vin, scalar1=w_off_sb[:, 0:1],
                    scalar2=b_off_sb[:, 0:1], op0=ALU.mult, op1=ALU.add)
            else:
                nc.vector.scalar_tensor_tensor(
                    out=ova, in0=vin, scalar=w_off_sb[:, tap:tap + 1],
                    in1=ova, op0=ALU.mult, op1=ALU.add)

        # transpose offsets to position-partition layout (batched index math)
        with tc.tile_pool(name="psT", bufs=2, space="PSUM") as psT, \
             tc.tile_pool(name="scr", bufs=1) as scr:
            offT = scr.tile([128, NPT, OFFC], FP32)
            for t in range(NPT):
                pso = psT.tile([128, OFFC], BF16, tag="pst")
                nc.tensor.transpose(pso[:], off_sb[:, t * 128:(t + 1) * 128],
                                    ident_b[:OFFC, :OFFC])
                nc.scalar.copy(offT[:, t, :], pso[:])

            dyv = offT[:].rearrange("p t (k two) -> p t k two", two=2)[:, :, :, 0]
            dxv = offT[:].rearrange("p t (k two) -> p t k two", two=2)[:, :, :, 1]
            py = scr.tile([128, NPT, K], FP32)
            px = scr.tile([128, NPT, K], FP32)
            nc.vector.scalar_tensor_tensor(out=py[:], in0=dyv, scalar=hh[:, 0:1],
                                           in1=by_f[:], op0=ALU.add, op1=ALU.add)
            nc.vector.scalar_tensor_tensor(out=px[:], in0=dxv, scalar=ww[:, 0:1],
                                           in1=bx_f[:], op0=ALU.add, op1=ALU.add)
            fyi = scr.tile([128, NPT, K], I32)
            fxi = scr.tile([128, NPT, K], I32)
            nc.vector.tensor_copy(fyi[:], py[:])
            nc.vector.tensor_copy(fxi[:], px[:])
            fy = scr.tile([128, NPT, K], FP32)
            fx = scr.tile([128, NPT, K], FP32)
            nc.vector.tensor_copy(fy[:], fyi[:])
            nc.vector.tensor_copy(fx[:], fxi[:])
            m = scr.tile([128, NPT, K], FP32)
            nc.vector.tensor_tensor(out=m[:], in0=fy[:], in1=py[:], op=ALU.is_gt)
            nc.vector.tensor_sub(out=fy[:], in0=fy[:], in1=m[:])
            nc.vector.tensor_tensor(out=m[:], in0=fx[:], in1=px[:], op=ALU.is_gt)
            nc.vector.tensor_sub(out=fx[:], in0=fx[:], in1=m[:])
            # residuals, k-major slots (rx, ry, rxry)
            wv = wts_sb[:].rearrange("p t (k s) -> p t k s", s=3)
            nc.vector.tensor_sub(out=wv[:, :, :, 0], in0=px[:], in1=fx[:])
            nc.vector.tensor_sub(out=wv[:, :, :, 1], in0=py[:], in1=fy[:])
            nc.vector.tensor_tensor(out=wv[:, :, :, 2], in0=wv[:, :, :, 0],
                                    in1=wv[:, :, :, 1], op=ALU.mult)
            r0f = scr.tile([128, NPT, K], FP32)
            nc.vector.scalar_tensor_tensor(out=r0f[:], in0=fy[:], scalar=float(G),
                                           in1=fx[:], op0=ALU.mult, op1=ALU.add)
            nc.vector.tensor_scalar(out=r0f[:], in0=r0f[:], scalar1=0.0,
                                    scalar2=float(RT - G - 2), op0=ALU.max,
                                    op1=ALU.min)
            # fold r0 [128q, t, k] -> idx16[p, t, 8k+r] = r0(16r+p%16, t, k)
            # via 8 one-hot select matmuls S_r[q,p] = (q == 16r + p%16)
            pm16_i = scr.tile([128, 128], I32)
            nc.gpsimd.iota(pm16_i[:], pattern=[[0, 8], [1, 16]], base=0,
                           channel_multiplier=0)
            pm16 = scr.tile([128, 128], FP32)
            nc.vector.tensor_copy(pm16[:], pm16_i[:])
            qvr_i = scr.tile([128, 8], I32)
            nc.gpsimd.iota(qvr_i[:], pattern=[[-16, 8]], base=0,
                           channel_multiplier=1)
            qvr = scr.tile([128, 8], FP32)
            nc.vector.tensor_copy(qvr[:], qvr_i[:])
            sel = scr.tile([128, 8, 128], FP32)
            r0flat = r0f[:].rearrange("p t k -> p (t k)")
            for r in range(8):
                nc.vector.tensor_scalar(out=sel[:, r, :], in0=pm16[:],
                                        scalar1=qvr[:, r:r + 1], scalar2=None,
                                        op0=ALU.is_equal)
            for r in range(8):
                ps = psT.tile([128, NPT * K], FP32, tag="pidx")
                nc.tensor.matmul(out=ps[:], lhsT=sel[:, r, :], rhs=r0flat,
                                 start=True, stop=True)
                nc.vector.tensor_copy(
                    idx16[:, :, :, r],
                    ps[:].rearrange("p (t k) -> p t k", k=K))
        ph1.close()

        # ---------------- phase 2: table build (all bf16) ----------------
        with tc.tile_pool(name="tblp", bufs=1) as tblp, \
             tc.tile_pool(name="psB", bufs=4, space="PSUM") as psB, \
             tc.tile_pool(name="evb", bufs=3) as evb:
            xbf = tblp.tile([128, CT, RT], BF16)
            nc.scalar.memzero(xbf[:])
            dbf = tblp.tile([128, CT, 3, RT], BF16)
            for ct in range(CT):
                nc.vector.tensor_copy(
                    xbf[:, ct, :ROWS].rearrange("p (h w) -> p h w", h=G, w=G)
                        [:, PAD:PAD + H, PAD:PAD + W],
                    x_sb[:, ct, :].rearrange("p (h w) -> p h w", h=H, w=W))
            for ct in range(CT):
                nc.vector.tensor_sub(out=dbf[:, ct, 0, 0:RT - 1],
                                     in0=xbf[:, ct, 1:RT], in1=xbf[:, ct, 0:RT - 1])
                nc.gpsimd.memset(dbf[:, ct, 0, RT - 1:RT], 0.0)
                nc.vector.tensor_sub(out=dbf[:, ct, 1, 0:RT - G],
                                     in0=xbf[:, ct, G:RT], in1=xbf[:, ct, 0:RT - G])
                nc.gpsimd.memset(dbf[:, ct, 1, RT - G:RT], 0.0)
                nc.vector.tensor_sub(out=dbf[:, ct, 2, 0:RT - G],
                                     in0=dbf[:, ct, 0, G:RT], in1=dbf[:, ct, 0, 0:RT - G])
                nc.gpsimd.memset(dbf[:, ct, 2, RT - G:RT], 0.0)

            for rt in range(NRT):
                tb = evb.tile([128, 4, C], BF16, tag="tb")
                for ct in range(CT):
                    ps = psB.tile([128, 4 * 128], BF16, tag="ps")
                    nc.tensor.transpose(ps[:, 0:128],
                                        xbf[:, ct, rt * 128:(rt + 1) * 128], ident_b[:])
                    for s in range(3):
                        nc.tensor.transpose(
                            ps[:, (s + 1) * 128:(s + 2) * 128],
                            dbf[:, ct, s, rt * 128:(rt + 1) * 128], ident_b[:])
                    # one grouped evac: psum [128, 512] -> tb strided slots
                    tbv = tb[:, :, ct * 128:(ct + 1) * 128]
                    psv = ps[:].rearrange("p (s c) -> p s c", s=4)
                    if (rt + ct) % 2 == 0:
                        nc.scalar.copy(tbv, psv)
                    else:
                        nc.vector.tensor_copy(tbv, psv)
                nc.sync.dma_start(out=table[rt * 128:(rt + 1) * 128, :], in_=tb[:])
        xs_stack.close()

        if MODE == "pre":
            with tc.tile_pool(name="zz", bufs=1) as zz:
                zt = zz.tile([128, HW], FP32)
                nc.vector.memset(zt[:], 0.0)
                for ot in range(2):
                    nc.sync.dma_start(out=out_d[ot * 128:(ot + 1) * 128, :], in_=zt[:])
            return nc

        # ---------------- phase 4: main loop ----------------
        outp = ctx.enter_context(tc.tile_pool(name="outp", bufs=1))
        out_sb = outp.tile([128, 2, HW], FP32)
        with tc.tile_pool(name="gat", bufs=int(os.environ.get("GBUFS", "3"))) as gat, \
             tc.tile_pool(name="smp", bufs=int(os.environ.get("SBUFS", "2"))) as smp, \
             tc.tile_pool(name="psS", bufs=4, space="PSUM") as psS, \
             tc.tile_pool(name="psO", bufs=2, space="PSUM") as psO:
            for t in range(NPT):
                g_sb = gat.tile([128, K, 4 * C], BF16, tag="g")
                nc.gpsimd.dma_gather(
                    out_ap=g_sb[:],
                    in_ap=table[:, :],
                    idxs_ap=idx16[:, t, :, :],
                    num_idxs=K * 128,
                    num_idxs_reg=K * 128,
                    elem_size=4 * C)
                samp = smp.tile([128, KT * 128], BF16, tag="s")
                for k in range(K):
                    av = samp[:, k * C:(k + 1) * C]
                    eng = nc.vector if k < int(os.environ.get('DVE_TAPS', '9')) else nc.gpsimd
                    eng.scalar_tensor_tensor(
                        out=av, in0=g_sb[:, k, C:2 * C],
                        scalar=wts_sb[:, t, 3 * k:3 * k + 1],
                        in1=g_sb[:, k, 0:C], op0=ALU.mult, op1=ALU.add)
                    eng.scalar_tensor_tensor(
                        out=av, in0=g_sb[:, k, 2 * C:3 * C],
                        scalar=wts_sb[:, t, 3 * k + 1:3 * k + 2],
                        in1=av, op0=ALU.mult, op1=ALU.add)
                    eng.scalar_tensor_tensor(
                        out=av, in0=g_sb[:, k, 3 * C:4 * C],
                        scalar=wts_sb[:, t, 3 * k + 2:3 * k + 3],
                        in1=av, op0=ALU.mult, op1=ALU.add)

                sampT = smp.tile([128, KT, 128], BF16, tag="st")
                for q in range(5):   # groups of 4 transposes -> one evac
                    n_in_g = 4 if q < 4 else 2
                    ps = psS.tile([128, 4 * 128], BF16, tag="pss")
                    for j in range(n_in_g):
                        kt = q * 4 + j
                        nc.tensor.transpose(ps[:, j * 128:(j + 1) * 128],
                                            samp[:, kt * 128:(kt + 1) * 128], ident_b[:])
                    nc.scalar.copy(sampT[:, q * 4:q * 4 + n_in_g, :],
                                   ps[:, :n_in_g * 128])
                for ot in range(2):
                    pso = psO.tile([128, 128], FP32, tag="po")
                    for kt in range(KT):
                        nc.tensor.matmul(out=pso[:],
                                         lhsT=w_defT[:, kt, ot * 128:(ot + 1) * 128],
                                         rhs=sampT[:, kt, :],
                                         start=(kt == 0), stop=(kt == KT - 1))
                    nc.scalar.copy(out_sb[:, ot, t * 128:(t + 1) * 128], pso[:])
            for ot in range(2):
                nc.sync.dma_start(out=out_d[ot * 128:(ot + 1) * 128, :],
                                  in_=out_sb[:, ot, :])
    return nc


_CACHE = {}


def _get_nc():
    if "nc" not in _CACHE:
        nc = build_nc()
        if not nc.is_finalized():
            nc.finalize()
        _CACHE["nc"] = nc
    return _CACHE["nc"]


def kernel(**inputs):
    from concourse import bass_utils
    x = np.ascontiguousarray(inputs["x"], dtype=np.float32)          # [8,256,64,64]
    w_adj = np.ascontiguousarray(inputs["w_adj"], dtype=np.float32).reshape(OFFC, C)
    b_adj = np.ascontiguousarray(inputs["b_adj"], dtype=np.float32).reshape(OFFC, 1)
    w_off = np.ascontiguousarray(inputs["w_off"], dtype=np.float32).reshape(OFFC, K)
    b_off = np.ascontiguousarray(inputs["b_off"], dtype=np.float32).reshape(OFFC, 1)
    w_def = np.ascontiguousarray(inputs["w_def"], dtype=np.float32).reshape(C, C * K)

    nc = _get_nc()
    in_maps = []
    for n in range(N):
        in_maps.append({
            "x": np.ascontiguousarray(x[n].reshape(C, HW)),
            "w_adj": w_adj, "b_adj": b_adj,
            "w_off": w_off, "b_off": b_off,
            "w_def": w_def,
        })
    res = bass_utils.run_bass_kernel_spmd(nc, in_maps, core_ids=list(range(N)))
    outs = [res.results[n]["out"].reshape(C, H, W) for n in range(N)]
    return np.stack(outs, axis=0)


if __name__ == "__main__":
    nc = build_nc()
    print("build ok")



# revision 9
# speedup vs baseline: 1.0987x; 1.0987x over previous
"""Deformable conv (nn_DeformConv) Trainium2 Bass kernel.

Strategy (per core = one batch of 8, data-parallel):
  1. 1x1 conv (PE) + depthwise 3x3 (DVE, shifted views) -> offsets [18, 4096]
  2. PE-transpose offsets to position-partition layout; batched per-position
     floor/residual math and flat gather index r0 into a zero-padded 72x72 grid.
  3. DRAM table [5248 rows, 1024] bf16, row r = [x[r] | Dx[r] | Dy[r] | Dxy[r]]
     (finite differences of zero-padded x). Bilinear sample ==
     x[r0] + rx*Dx[r0] + ry*Dy[r0] + rx*ry*Dxy[r0] (exact, incl. OOB zeroing).
  4. Per 128-position tile: indirect row-gathers (per tap), wide broadcast-AP
     multiply + 3 adds for the combine, PE-transpose sampled into 18 ck-tiles,
     PSUM-accumulated matmul against w_def (bf16), DMA out.
"""
import os
import numpy as np
from contextlib import ExitStack

import concourse.bass as bass
import concourse.mybir as mybir
import concourse.tile as tile
from concourse import bacc as _bacc
from concourse.bass import IndirectOffsetOnAxis
from concourse.masks import make_identity

FP32 = mybir.dt.float32
BF16 = mybir.dt.bfloat16
I32 = mybir.dt.int32
I16 = mybir.dt.int16

N, C, H, W = 8, 256, 64, 64
HW = H * W                    # 4096
K = 9
OFFC = 18
PAD = 4
G = H + 2 * PAD               # 72
ROWS = G * G                  # 5184
RT = 5248                     # rows padded to 41*128
NRT = RT // 128               # 41
NPT = HW // 128               # 32 position tiles
CT = C // 128                 # 2 channel tiles
KT = (C * K) // 128           # 18 contraction tiles
ALU = mybir.AluOpType
AF = mybir.ActivationFunctionType

MODE = os.environ.get("MODE", "full")   # full | pre
# NOTE: multi-offset indirect DMA fails at runtime on HW; keep per-tap gathers.
MERGE_GATHER = os.environ.get("MERGE_GATHER", "0") == "1"


def build_nc():
    nc = _bacc.Bacc()
    x_d = nc.dram_tensor("x", [C, HW], FP32, kind="ExternalInput")
    w_adj_d = nc.dram_tensor("w_adj", [OFFC, C], FP32, kind="ExternalInput")
    b_adj_d = nc.dram_tensor("b_adj", [OFFC, 1], FP32, kind="ExternalInput")
    w_off_d = nc.dram_tensor("w_off", [OFFC, K], FP32, kind="ExternalInput")
    b_off_d = nc.dram_tensor("b_off", [OFFC, 1], FP32, kind="ExternalInput")
    w_def_d = nc.dram_tensor("w_def", [C, C * K], FP32, kind="ExternalInput")
    out_d = nc.dram_tensor("out", [C, HW], FP32, kind="ExternalOutput")

    with tile.TileContext(nc) as tc, ExitStack() as ctx:
        pers = ctx.enter_context(tc.tile_pool(name="pers", bufs=1))
        dram = ctx.enter_context(tc.tile_pool(name="dram", bufs=1, space="DRAM"))

        table = dram.tile([RT, 4 * C], BF16)

        ident_f = pers.tile([128, 128], FP32)
        make_identity(nc, ident_f[:])
        ident_b = pers.tile([128, 128], BF16)
        nc.vector.tensor_copy(ident_b[:], ident_f[:])

        # per-partition constants: hh = p//64 (0/1), ww = p%64
        iota_p = pers.tile([128, 1], I32)
        nc.gpsimd.iota(iota_p[:], pattern=[[0, 1]], base=0, channel_multiplier=1)
        pf = pers.tile([128, 1], FP32)
        nc.vector.tensor_copy(pf[:], iota_p[:])
        hh = pers.tile([128, 1], FP32)
        nc.vector.tensor_scalar(out=hh[:], in0=pf[:], scalar1=64.0, scalar2=None,
                                op0=ALU.is_ge)
        ww = pers.tile([128, 1], FP32)
        nc.vector.scalar_tensor_tensor(out=ww[:], in0=hh[:], scalar=-64.0,
                                       in1=pf[:], op0=ALU.mult, op1=ALU.add)

        # batched base ramps over (t, k): by = 2t + ki + (PAD-1), bx = kj + (PAD-1)
        by_i = pers.tile([128, NPT, K], I32)
        nc.gpsimd.iota(by_i[:], pattern=[[2, NPT], [1, 3], [0, 3]], base=PAD - 1,
                       channel_multiplier=0)
        bx_i = pers.tile([128, NPT, K], I32)
        nc.gpsimd.iota(bx_i[:], pattern=[[0, NPT], [0, 3], [1, 3]], base=PAD - 1,
                       channel_multiplier=0)
        by_f = pers.tile([128, NPT, K], FP32)
        nc.vector.tensor_copy(by_f[:], by_i[:])
        bx_f = pers.tile([128, NPT, K], FP32)
        nc.vector.tensor_copy(bx_f[:], bx_i[:])

        w_defT = pers.tile([128, KT, 2 * 128], BF16)   # [ck-part, kt, o]
        # gather indices, 16-partition-wrapped for dma_gather:
        # idx16[p, t, 8k+r] = r0(q=16r+p%16, t, k)
        idx16 = pers.tile([128, NPT, K, 8], I16)
        wts_sb = pers.tile([128, NPT, K * 3], FP32)    # k-major (rx, ry, rxry)

        # ---------------- phase 3: w_def transpose (overlaps phase 1) ----------------
        xs_stack = ExitStack()
        xp = xs_stack.enter_context(tc.tile_pool(name="xp", bufs=1))
        with tc.tile_pool(name="psW", bufs=4, space="PSUM") as psW:
            w_def_sb = xp.tile([128, 2, C * K], FP32)
            for ot in range(2):
                nc.sync.dma_start(out=w_def_sb[:, ot, :],
                                  in_=w_def_d[ot * 128:(ot + 1) * 128, :])
            for kt in range(KT):
                k = kt // 2
                chalf = kt % 2
                for ot in range(2):
                    ps = psW.tile([128, 128], FP32, tag="psw")
                    src = w_def_sb[:, ot, :].rearrange("p (c k) -> p k c", k=K) \
                        [:, k, chalf * 128:(chalf + 1) * 128]
                    nc.tensor.transpose(ps[:], src, ident_f[:])
                    nc.scalar.copy(w_defT[:, kt, ot * 128:ot * 128 + 128], ps[:])

        # ---------------- phase 1: offsets pipeline ----------------
        x_sb = xp.tile([128, CT, HW], FP32)
        for ct in range(CT):
            nc.sync.dma_start(out=x_sb[:, ct, :], in_=x_d[ct * 128:(ct + 1) * 128, :])

        ph1 = ExitStack()
        offp = ph1.enter_context(tc.tile_pool(name="offp", bufs=1))
        psA = ph1.enter_context(tc.tile_pool(name="psA", bufs=2, space="PSUM"))

        w_adjT = offp.tile([128, CT, OFFC], FP32)
        for ct in range(CT):
            nc.sync.dma_start(
                out=w_adjT[:, ct, :],
                in_=w_adj_d.rearrange("o c -> c o")[ct * 128:(ct + 1) * 128, :])
        b_adj_sb = offp.tile([OFFC, 1], FP32)
        nc.sync.dma_start(out=b_adj_sb[:], in_=b_adj_d[:, :])
        w_off_sb = offp.tile([OFFC, K], FP32)
        nc.sync.dma_start(out=w_off_sb[:], in_=w_off_d[:, :])
        b_off_sb = offp.tile([OFFC, 1], FP32)
        nc.sync.dma_start(out=b_off_sb[:], in_=b_off_d[:, :])

        # 1x1 conv -> x_chan (padded 66x66 for the depthwise conv)
        GC = H + 2   # 66
        xch_pad = offp.tile([OFFC, GC * GC], BF16)
        nc.scalar.memzero(xch_pad[:])
        xch_v = xch_pad[:].rearrange("p (h w) -> p h w", h=GC, w=GC)
        for pch in range(8):
            ps = psA.tile([OFFC, 512], FP32)
            for ct in range(CT):
                nc.tensor.matmul(out=ps[:], lhsT=w_adjT[:, ct, :],
                                 rhs=x_sb[:, ct, pch * 512:(pch + 1) * 512],
                                 start=(ct == 0), stop=(ct == CT - 1))
            nc.scalar.activation(
                out=xch_v[:, 1 + pch * 8:1 + pch * 8 + 8, 1:1 + W],
                in_=ps[:].rearrange("p (h w) -> p h w", h=8, w=W),
                func=AF.Identity, bias=b_adj_sb[:], scale=1.0)

        # depthwise 3x3 -> offsets [18, 4096] (DVE chain, bf16)
        off_sb = offp.tile([OFFC, HW], BF16)
        ova = off_sb[:].rearrange("p (h w) -> p h w", h=H, w=W)
        for tap in range(K):
            di, dj = tap // 3, tap % 3
            vin = xch_v[:, di:di + H, dj:dj + W]
            if tap == 0:
                nc.vector.tensor_scalar(
                    out=ova, in0=vin, scalar1=w_off_sb[:, 0:1],
                    scalar2=b_off_sb[:, 0:1], op0=ALU.mult, op1=ALU.add)
            else:
                nc.vector.scalar_tensor_tensor(
                    out=ova, in0=vin, scalar=w_off_sb[:, tap:tap + 1],
                    in1=ova, op0=ALU.mult, op1=ALU.add)

        # transpose offsets to position-partition layout (batched index math)
        with tc.tile_pool(name="psT", bufs=2, space="PSUM") as psT, \
             tc.tile_pool(name="scr", bufs=1) as scr:
            offT = scr.tile([128, NPT, OFFC], FP32)
            for t in range(NPT):
                pso = psT.tile([128, OFFC], BF16, tag="pst")
                nc.tensor.transpose(pso[:], off_sb[:, t * 128:(t + 1) * 128],
                                    ident_b[:OFFC, :OFFC])
                nc.scalar.copy(offT[:, t, :], pso[:])

            dyv = offT[:].rearrange("p t (k two) -> p t k two", two=2)[:, :, :, 0]
            dxv = offT[:].rearrange("p t (k two) -> p t k two", two=2)[:, :, :, 1]
            py = scr.tile([128, NPT, K], FP32)
            px = scr.tile([128, NPT, K], FP32)
            nc.vector.scalar_tensor_tensor(out=py[:], in0=dyv, scalar=hh[:, 0:1],
                                           in1=by_f[:], op0=ALU.add, op1=ALU.add)
            nc.vector.scalar_tensor_tensor(out=px[:], in0=dxv, scalar=ww[:, 0:1],
                                           in1=bx_f[:], op0=ALU.add, op1=ALU.add)
            fyi = scr.tile([128, NPT, K], I32)
            fxi = scr.tile([128, NPT, K], I32)
            nc.vector.tensor_copy(fyi[:], py[:])
            nc.vector.tensor_copy(fxi[:], px[:])
            fy = scr.tile([128, NPT, K], FP32)
            fx = scr.tile([128, NPT, K], FP32)
            nc.vector.tensor_copy(fy[:], fyi[:])
            nc.vector.tensor_copy(fx[:], fxi[:])
            m = scr.tile([128, NPT, K], FP32)
            nc.vector.tensor_tensor(out=m[:], in0=fy[:], in1=py[:], op=ALU.is_gt)
            nc.vector.tensor_sub(out=fy[:], in0=fy[:], in1=m[:])
            nc.vector.tensor_tensor(out=m[:], in0=fx[:], in1=px[:], op=ALU.is_gt)
            nc.vector.tensor_sub(out=fx[:], in0=fx[:], in1=m[:])
            # residuals, k-major slots (rx, ry, rxry)
            wv = wts_sb[:].rearrange("p t (k s) -> p t k s", s=3)
            nc.vector.tensor_sub(out=wv[:, :, :, 0], in0=px[:], in1=fx[:])
            nc.vector.tensor_sub(out=wv[:, :, :, 1], in0=py[:], in1=fy[:])
            nc.vector.tensor_tensor(out=wv[:, :, :, 2], in0=wv[:, :, :, 0],
                                    in1=wv[:, :, :, 1], op=ALU.mult)
            r0f = scr.tile([128, NPT, K], FP32)
            nc.vector.scalar_tensor_tensor(out=r0f[:], in0=fy[:], scalar=float(G),
                                           in1=fx[:], op0=ALU.mult, op1=ALU.add)
            nc.vector.tensor_scalar(out=r0f[:], in0=r0f[:], scalar1=0.0,
                                    scalar2=float(RT - G - 2), op0=ALU.max,
                                    op1=ALU.min)
            # fold r0 [128q, t, k] -> idx16[p, t, 8k+r] = r0(16r+p%16, t, k)
            # via 8 one-hot select matmuls S_r[q,p] = (q == 16r + p%16)
            pm16_i = scr.tile([128, 128], I32)
            nc.gpsimd.iota(pm16_i[:], pattern=[[0, 8], [1, 16]], base=0,
                           channel_multiplier=0)
            pm16 = scr.tile([128, 128], FP32)
            nc.vector.tensor_copy(pm16[:], pm16_i[:])
            qvr_i = scr.tile([128, 8], I32)
            nc.gpsimd.iota(qvr_i[:], pattern=[[-16, 8]], base=0,
                           channel_multiplier=1)
            qvr = scr.tile([128, 8], FP32)
            nc.vector.tensor_copy(qvr[:], qvr_i[:])
            sel = scr.tile([128, 8, 128], FP32)
            r0flat = r0f[:].rearrange("p t k -> p (t k)")
            for r in range(8):
                nc.vector.tensor_scalar(out=sel[:, r, :], in0=pm16[:],
                                        scalar1=qvr[:, r:r + 1], scalar2=None,
                                        op0=ALU.is_equal)
            for r in range(8):
                ps = psT.tile([128, NPT * K], FP32, tag="pidx")
                nc.tensor.matmul(out=ps[:], lhsT=sel[:, r, :], rhs=r0flat,
                                 start=True, stop=True)
                nc.vector.tensor_copy(
                    idx16[:, :, :, r],
                    ps[:].rearrange("p (t k) -> p t k", k=K))
        ph1.close()

        # ---------------- phase 2: table build (all bf16) ----------------
        with tc.tile_pool(name="tblp", bufs=1) as tblp, \
             tc.tile_pool(name="psB", bufs=4, space="PSUM") as psB, \
             tc.tile_pool(name="evb", bufs=3) as evb:
            xbf = tblp.tile([128, CT, RT], BF16)
            nc.scalar.memzero(xbf[:])
            dbf = tblp.tile([128, CT, 3, RT], BF16)
            for ct in range(CT):
                nc.vector.tensor_copy(
                    xbf[:, ct, :ROWS].rearrange("p (h w) -> p h w", h=G, w=G)
                        [:, PAD:PAD + H, PAD:PAD + W],
                    x_sb[:, ct, :].rearrange("p (h w) -> p h w", h=H, w=W))
            for ct in range(CT):
                nc.vector.tensor_sub(out=dbf[:, ct, 0, 0:RT - 1],
                                     in0=xbf[:, ct, 1:RT], in1=xbf[:, ct, 0:RT - 1])
                nc.gpsimd.memset(dbf[:, ct, 0, RT - 1:RT], 0.0)
                nc.vector.tensor_sub(out=dbf[:, ct, 1, 0:RT - G],
                                     in0=xbf[:, ct, G:RT], in1=xbf[:, ct, 0:RT - G])
                nc.gpsimd.memset(dbf[:, ct, 1, RT - G:RT], 0.0)
                nc.vector.tensor_sub(out=dbf[:, ct, 2, 0:RT - G],
                                     in0=dbf[:, ct, 0, G:RT], in1=dbf[:, ct, 0, 0:RT - G])
                nc.gpsimd.memset(dbf[:, ct, 2, RT - G:RT], 0.0)

            for rt in range(NRT):
                tb = evb.tile([128, 4, C], BF16, tag="tb")
                for ct in range(CT):
                    ps = psB.tile([128, 4 * 128], BF16, tag="ps")
                    nc.tensor.transpose(ps[:, 0:128],
                                        xbf[:, ct, rt * 128:(rt + 1) * 128], ident_b[:])
                    for s in range(3):
                        nc.tensor.transpose(
                            ps[:, (s + 1) * 128:(s + 2) * 128],
                            dbf[:, ct, s, rt * 128:(rt + 1) * 128], ident_b[:])
                    # one grouped evac: psum [128, 512] -> tb strided slots
                    tbv = tb[:, :, ct * 128:(ct + 1) * 128]
                    psv = ps[:].rearrange("p (s c) -> p s c", s=4)
                    if (rt + ct) % 2 == 0:
                        nc.scalar.copy(tbv, psv)
                    else:
                        nc.vector.tensor_copy(tbv, psv)
                nc.sync.dma_start(out=table[rt * 128:(rt + 1) * 128, :], in_=tb[:])
        xs_stack.close()

        if MODE == "pre":
            with tc.tile_pool(name="zz", bufs=1) as zz:
                zt = zz.tile([128, HW], FP32)
                nc.vector.memset(zt[:], 0.0)
                for ot in range(2):
                    nc.sync.dma_start(out=out_d[ot * 128:(ot + 1) * 128, :], in_=zt[:])
            return nc

        # ---------------- phase 4: main loop ----------------
        outp = ctx.enter_context(tc.tile_pool(name="outp", bufs=1))
        out_sb = outp.tile([128, 2, HW], FP32)
        with tc.tile_pool(name="gat", bufs=int(os.environ.get("GBUFS", "3"))) as gat, \
             tc.tile_pool(name="smp", bufs=int(os.environ.get("SBUFS", "2"))) as smp, \
             tc.tile_pool(name="psS", bufs=1, space="PSUM") as psS, \
             tc.tile_pool(name="psO", bufs=2, space="PSUM") as psO:
            for t in range(NPT):
                g_sb = gat.tile([128, K, 4 * C], BF16, tag="g")
                nc.gpsimd.dma_gather(
                    out_ap=g_sb[:],
                    in_ap=table[:, :],
                    idxs_ap=idx16[:, t, :, :],
                    num_idxs=K * 128,
                    num_idxs_reg=K * 128,
                    elem_size=4 * C)
                # per-tap diag(coef) matrices, built on DVE (4x mode)
                diag = smp.tile([128, K * 3, 128], BF16, tag="d")
                for k in range(K):
                    for s in range(3):
                        nc.vector.tensor_scalar(
                            out=diag[:, 3 * k + s, :], in0=ident_b[:],
                            scalar1=wts_sb[:, t, 3 * k + s:3 * k + s + 1],
                            scalar2=None, op0=ALU.mult)
                # sampT[c, q] accumulated transposed in PSUM:
                #   psumT = x^T + (rx*Dx)^T + (ry*Dy)^T + (rxy*Dxy)^T
                # via out[c,q] = sum_p lhsT[p,c]*rhs[p,q], rhs in {I, diag}
                sampT = smp.tile([128, KT, 128], BF16, tag="st")
                for q in range(5):   # groups of 4 kt-slots -> one evac
                    n_in_g = 4 if q < 4 else 2
                    ps = psS.tile([128, 4, 128], FP32, tag=f"stp{q}")
                    for j in range(n_in_g):
                        kt = q * 4 + j
                        k, ct = kt // 2, kt % 2
                        for s in range(4):   # x, Dx, Dy, Dxy
                            lhsT = g_sb[:, k, s * C + ct * 128:s * C + ct * 128 + 128]
                            rhs = ident_b[:] if s == 0 else diag[:, 3 * k + s - 1, :]
                            nc.tensor.matmul(out=ps[:, j, :], lhsT=lhsT, rhs=rhs,
                                             start=(s == 0), stop=(s == 3))
                    if q % 2 == 0:
                        nc.scalar.copy(sampT[:, q * 4:q * 4 + n_in_g, :],
                                       ps[:, :n_in_g, :])
                    else:
                        nc.vector.tensor_copy(sampT[:, q * 4:q * 4 + n_in_g, :],
                                              ps[:, :n_in_g, :])
                for ot in range(2):
                    pso = psO.tile([128, 128], FP32, tag="po")
                    for kt in range(KT):
                        nc.tensor.matmul(out=pso[:],
                                         lhsT=w_defT[:, kt, ot * 128:(ot + 1) * 128],
                                         rhs=sampT[:, kt, :],
                                         start=(kt == 0), stop=(kt == KT - 1))
                    nc.scalar.copy(out_sb[:, ot, t * 128:(t + 1) * 128], pso[:])
            for ot in range(2):
                nc.sync.dma_start(out=out_d[ot * 128:(ot + 1) * 128, :],
                                  in_=out_sb[:, ot, :])
    return nc


_CACHE = {}


def _get_nc():
    if "nc" not in _CACHE:
        nc = build_nc()
        if not nc.is_finalized():
            nc.finalize()
        _CACHE["nc"] = nc
    return _CACHE["nc"]


def kernel(**inputs):
    from concourse import bass_utils
    x = np.ascontiguousarray(inputs["x"], dtype=np.float32)          # [8,256,64,64]
    w_adj = np.ascontiguousarray(inputs["w_adj"], dtype=np.float32).reshape(OFFC, C)
    b_adj = np.ascontiguousarray(inputs["b_adj"], dtype=np.float32).reshape(OFFC, 1)
    w_off = np.ascontiguousarray(inputs["w_off"], dtype=np.float32).reshape(OFFC, K)
    b_off = np.ascontiguousarray(inputs["b_off"], dtype=np.float32).reshape(OFFC, 1)
    w_def = np.ascontiguousarray(inputs["w_def"], dtype=np.float32).reshape(C, C * K)

    nc = _get_nc()
    in_maps = []
    for n in range(N):
        in_maps.append({
            "x": np.ascontiguousarray(x[n].reshape(C, HW)),
            "w_adj": w_adj, "b_adj": b_adj,
            "w_off": w_off, "b_off": b_off,
            "w_def": w_def,
        })
    res = bass_utils.run_bass_kernel_spmd(nc, in_maps, core_ids=list(range(N)))
    outs = [res.results[n]["out"].reshape(C, H, W) for n in range(N)]
    return np.stack(outs, axis=0)


if __name__ == "__main__":
    nc = build_nc()
    print("build ok")

